# revision 19
# baseline (speedup 1.0000x reference)
"""Trainium2 Bass kernel for nn_AttentionModule (gnn_message_passing).

Takes FULL inputs, shards batch dim across 8 NeuronCores (pure data
parallel), runs a hand-written Bass/Tile kernel per core, gathers the
full output.

Self-contained: all shapes/constants hardcoded.
"""

import numpy as np
import ml_dtypes

import concourse.bass as bass
import concourse.bacc as bacc_mod
import concourse.tile as tile
from concourse import mybir
from concourse.bass_utils import run_bass_kernel_spmd

# ---------------- problem constants (hardcoded) ----------------
B, K, C, CI = 4096, 17, 256, 64
NCORES = 8
BC = B // NCORES            # 512 batches per core
R = BC * K                  # 8704 rows per core
NT = BC // 128              # 4 batch sub-chunks of 128
GROUPS = [[0, 1, 2, 3, 4], [5, 7, 9], [6, 8, 10], [11, 13, 15], [12, 14, 16]]
G = len(GROUPS)
KORDER = [k for g in GROUPS for k in g]          # group-sorted keypoint order
GSIZES = [len(g) for g in GROUPS]
GOFF = [0, 5, 8, 11, 14, 17]                     # group offsets in k'-space
GRP_OF_KP = [gi for gi, g in enumerate(GROUPS) for _ in g]
EDGES = [(i, j) for i in range(G) for j in range(G) if j != i]  # 20 directed
BN_EPS = 1e-5

F32 = mybir.dt.float32
F32R = mybir.dt.float32r
BF16 = mybir.dt.bfloat16
AFT = mybir.ActivationFunctionType

# number of final-gate multiplies routed to GpSimd instead of DVE (tuning)
N_GPSIMD_GATE = 4


def build_nc(act=AFT.Silu, repeat=1):
    nc = bacc_mod.Bacc()
    x_h = nc.declare_dram_parameter("x", [R, C], F32R, isOutput=False)
    w1_h = nc.declare_dram_parameter("w1", [128, 2 * CI], F32R, isOutput=False)
    b1_h = nc.declare_dram_parameter("b1", [128, 1], F32, isOutput=False)
    wpe_h = nc.declare_dram_parameter("wpe", [CI, 10 * CI], BF16, isOutput=False)
    b2_h = nc.declare_dram_parameter("b2", [128, 1], F32, isOutput=False)
    wag_h = nc.declare_dram_parameter("wag", [CI + 1, C], BF16, isOutput=False)
    id_h = nc.declare_dram_parameter("ident", [128, 128], F32R, isOutput=False)
    ones_h = nc.declare_dram_parameter("ones", [1, G * BC], BF16, isOutput=False)
    out_h = nc.declare_dram_parameter("out", [R, C], F32, isOutput=True)

    # row r of x = b*K + k with b = t*128 + p  ->  view [k, p, t, c]
    x_r = x_h[:].rearrange("(t p k) c -> k p t c", t=NT, p=128, k=K)
    out_r = out_h[:].rearrange("(t p k) c -> k p t c", t=NT, p=128, k=K)

    import contextlib
    with tile.TileContext(nc) as tc:
        rep_ctx = tc.For_i(0, repeat, 1) if repeat > 1 else contextlib.nullcontext()
        with (
            tc.tile_pool(name="consts", bufs=1) as consts,
            tc.tile_pool(name="xres", bufs=K + 1) as xres,
            tc.tile_pool(name="xt", bufs=4) as xtp,
            tc.tile_pool(name="bigs", bufs=1) as bigs,
            tc.tile_pool(name="outp", bufs=4) as outp,
            tc.tile_pool(name="pstr", bufs=2, space="PSUM") as pstr,
            tc.tile_pool(name="psmm", bufs=2, space="PSUM") as psmm,
            tc.tile_pool(name="pspe", bufs=2, space="PSUM") as pspe,
            tc.tile_pool(name="psat", bufs=2, space="PSUM") as psat,
            rep_ctx,
        ):
            # ---- constants ----
            w1_sb = consts.tile([128, 2 * CI], F32R)
            nc.sync.dma_start(out=w1_sb, in_=w1_h[:])
            b1_sb = consts.tile([128, 1], F32)
            nc.sync.dma_start(out=b1_sb, in_=b1_h[:])
            wpe_sb = consts.tile([CI, 10 * CI], BF16)
            nc.sync.dma_start(out=wpe_sb, in_=wpe_h[:])
            b2_sb = consts.tile([128, 1], F32)
            nc.sync.dma_start(out=b2_sb, in_=b2_h[:])
            wag_sb = consts.tile([CI + 1, C], BF16)
            nc.sync.dma_start(out=wag_sb, in_=wag_h[:])
            ident_r = consts.tile([128, 128], F32R)
            nc.sync.dma_start(out=ident_r, in_=id_h[:])

            # ---- big SBUF tensors ----
            xd_sb = bigs.tile([CI, K * BC], BF16)
            sums_sb = bigs.tile([CI, G * BC], BF16)
            pe_sb = bigs.tile([CI, 20 * BC], BF16)
            agg_sb = bigs.tile([CI + 1, G * BC], BF16)
            att_sb = bigs.tile([128, 20 * C], F32)

            # ---- phase A: load, transpose, down-proj, silu1 ----
            xk_tiles = []
            for kp in range(K):
                xk = xres.tile([128, NT * C], F32R)
                nc.sync.dma_start(
                    out=xk.rearrange("p (t c) -> p t c", c=C),
                    in_=x_r[KORDER[kp]],
                )
                xk_tiles.append(xk)

            for kp in range(K):
                ps1 = psmm.tile([CI, BC], F32)
                xk = xk_tiles[kp]
                for ch in range(2):
                    pst = pstr.tile([128, BC], F32R)
                    for t in range(NT):
                        nc.tensor.transpose(
                            out=pst[:, t * 128:(t + 1) * 128],
                            in_=xk[:, t * C + ch * 128: t * C + ch * 128 + 128],
                            identity=ident_r,
                        )
                    xt_t = xtp.tile([128, BC], F32R)
                    nc.vector.tensor_copy(out=xt_t, in_=pst)
                    nc.tensor.matmul(
                        out=ps1,
                        lhsT=w1_sb[:, ch * CI:(ch + 1) * CI],
                        rhs=xt_t,
                        start=(ch == 0),
                        stop=(ch == 1),
                    )
                nc.scalar.activation(
                    out=xd_sb[:, kp * BC:(kp + 1) * BC], in_=ps1,
                    func=act, bias=b1_sb[0:CI],
                )

            # ---- phase B: group sums (scale folded into pe weights) ----
            def xd_slab(kp):
                return xd_sb[:, kp * BC:(kp + 1) * BC]

            for g in range(G):
                sl = slice(g * BC, (g + 1) * BC)
                kps = list(range(GOFF[g], GOFF[g + 1]))
                nc.vector.tensor_add(
                    out=sums_sb[:, sl], in0=xd_slab(kps[0]), in1=xd_slab(kps[1])
                )
                for kp in kps[2:]:
                    nc.vector.tensor_add(
                        out=sums_sb[:, sl], in0=sums_sb[:, sl], in1=xd_slab(kp)
                    )

            # ---- phase C: edge conv via accumulating matmuls + silu2 ----
            for e in range(20):
                i, j = EDGES[e]
                psq = pspe.tile([CI, BC], F32)
                nc.tensor.matmul(
                    out=psq,
                    lhsT=wpe_sb[:, i * CI:(i + 1) * CI],
                    rhs=sums_sb[:, i * BC:(i + 1) * BC],
                    start=True, stop=False,
                )
                nc.tensor.matmul(
                    out=psq,
                    lhsT=wpe_sb[:, (5 + j) * CI:(6 + j) * CI],
                    rhs=sums_sb[:, j * BC:(j + 1) * BC],
                    start=False, stop=True,
                )
                nc.scalar.activation(
                    out=pe_sb[:, e * BC:(e + 1) * BC], in_=psq,
                    func=act, bias=b2_sb[0:CI],
                )

            # ---- phase D: scatter-add over target nodes ----
            nc.sync.dma_start(out=agg_sb[CI:CI + 1, :], in_=ones_h[:])  # bias trick
            def pe_slab(e):
                return pe_sb[:, e * BC:(e + 1) * BC]

            for i in range(G):
                sl = slice(i * BC, (i + 1) * BC)
                es = [4 * i, 4 * i + 1, 4 * i + 2, 4 * i + 3]
                nc.vector.tensor_add(
                    out=agg_sb[0:CI, sl], in0=pe_slab(es[0]), in1=pe_slab(es[1])
                )
                for e in es[2:]:
                    nc.vector.tensor_add(
                        out=agg_sb[0:CI, sl], in0=agg_sb[0:CI, sl], in1=pe_slab(e)
                    )

            # ---- phase E: attention matmul + sigmoid ----
            for p in range(10):
                psa = psat.tile([128, 2 * C], F32)
                for half in range(2):
                    t = p * 2 + half
                    nc.tensor.matmul(
                        out=psa[:, half * C:(half + 1) * C],
                        lhsT=agg_sb[:, t * 128:(t + 1) * 128],
                        rhs=wag_sb,
                        start=True, stop=True,
                    )
                nc.scalar.activation(
                    out=att_sb[:, p * 2 * C:(p + 1) * 2 * C], in_=psa, func=AFT.Sigmoid
                )

            # ---- phase F: gate + store ----
            for idx in range(K):
                g = GRP_OF_KP[idx]
                xk = xk_tiles[idx]
                ot = outp.tile([128, NT * C], F32)
                eng = nc.gpsimd if idx < N_GPSIMD_GATE else nc.vector
                eng.tensor_mul(
                    out=ot, in0=xk.bitcast(F32),
                    in1=att_sb[:, g * NT * C:(g + 1) * NT * C],
                )
                nc.sync.dma_start(
                    out=out_r[KORDER[idx]],
                    in_=ot.rearrange("p (t c) -> p t c", c=C),
                )

    nc.compile()
    return nc


def _prep_weights(W_down, b_down, bn1_scale, bn1_bias, bn1_mean, bn1_var,
                  W_conv, bn2_scale, bn2_bias, bn2_mean, bn2_var, W_agg, b_agg):
    f64 = np.float64
    a1 = bn1_scale.astype(f64) / np.sqrt(bn1_var.astype(f64) + BN_EPS)
    W1f = W_down.astype(f64) * a1[None, :]                      # [256, 64]
    b1f = (b_down.astype(f64) - bn1_mean) * a1 + bn1_bias       # [64]

    a2 = bn2_scale.astype(f64) / np.sqrt(bn2_var.astype(f64) + BN_EPS)
    Wc = W_conv.astype(f64) * a2[:, None]                       # [64, 128]
    b2f = bn2_bias.astype(f64) - bn2_mean * a2                  # [64]
    W1, W2 = Wc[:, :CI], Wc[:, CI:]
    Wp = W1 - W2

    # w1 sbuf layout: [128, 2*CI], col-block ch = W1f[ch*128:(ch+1)*128, :]
    w1 = np.concatenate([W1f[:128, :], W1f[128:, :]], axis=1).astype(np.float32)
    b1 = np.tile(b1f.reshape(CI, 1), (2, 1)).astype(np.float32)  # [128, 1]

    # wpe: [64, 10*64]: blocks 0..4 = Wp.T/|g_i|, 5..9 = W2.T/|g_j|
    blocks = [Wp.T / GSIZES[i] for i in range(G)] + [W2.T / GSIZES[j] for j in range(G)]
    wpe = np.concatenate(blocks, axis=1).astype(ml_dtypes.bfloat16)
    b2 = np.tile(b2f.reshape(CI, 1), (2, 1)).astype(np.float32)

    wag = np.concatenate(
        [W_agg.astype(f64), b_agg.astype(f64)[None, :]], axis=0
    ).astype(ml_dtypes.bfloat16)                                # [65, 256]
    return w1, b1, wpe, b2, wag


_NC_CACHE = {}


def _run(inputs, trace=False, trace_kwargs=None):
    x = np.ascontiguousarray(np.asarray(inputs["x_bk_c"], dtype=np.float32))
    assert x.shape == (B * K, C), x.shape
    w1, b1, wpe, b2, wag = _prep_weights(
        np.asarray(inputs["W_down"]), np.asarray(inputs["b_down"]),
        np.asarray(inputs["bn1_scale"]), np.asarray(inputs["bn1_bias"]),
        np.asarray(inputs["bn1_mean"]), np.asarray(inputs["bn1_var"]),
        np.asarray(inputs["W_conv"]),
        np.asarray(inputs["bn2_scale"]), np.asarray(inputs["bn2_bias"]),
        np.asarray(inputs["bn2_mean"]), np.asarray(inputs["bn2_var"]),
        np.asarray(inputs["W_agg"]), np.asarray(inputs["b_agg"]),
    )

    if "nc" not in _NC_CACHE:
        _NC_CACHE["nc"] = build_nc()
    nc = _NC_CACHE["nc"]

    in_maps = []
    for c in range(NCORES):
        in_maps.append({
            "x": np.ascontiguousarray(x[c * R:(c + 1) * R]),
            "w1": w1, "b1": b1, "wpe": wpe, "b2": b2, "wag": wag,
            "ident": np.eye(128, dtype=np.float32),
            "ones": np.ones((1, G * BC), dtype=ml_dtypes.bfloat16),
        })
    kw = {}
    if trace:
        kw["trace"] = True
        if trace_kwargs:
            kw["trace_kwargs"] = trace_kwargs
    res = run_bass_kernel_spmd(nc, in_maps, core_ids=list(range(NCORES)), **kw)
    out = np.concatenate([r["out"] for r in res.results], axis=0)
    return out, res


def kernel(**inputs) -> np.ndarray:
    out, _ = _run(inputs)
    return out
